# revision 3
# baseline (speedup 1.0000x reference)
"""Trainium2 Bass kernel for nn_ComplexQuantumLayer (10-qubit circuit, batch 2048).

Math: the circuit after the RX AngleEmbedding is a fixed unitary U (depends only
on `weights`), and the embedded state is a Kronecker product
  psi0[b] = (-i)^popcount(j) * m[b, j],   m[b] = kron_q [cos(x_bq/2), sin(x_bq/2)].
Folding the phase into W = diag(phase) @ U^T gives  psi = m @ W  with m REAL.
Per sample the device does two real (1024,1024) matvecs (fp16 operands, fp32
PSUM), |psi|^2, and the ten PauliZ sums as one more small matmul against a
+/-1 mask matrix.

v2 design (per core, 256 samples):
  - The host sends m already TRANSPOSED (amp-major): mt[p, k, b] = m[b, k*128+p]
    (512KB fp16). This removes the on-device Kronecker tree, all PE transposes
    and PSUM->SBUF copies of v1; the PE runs nothing but the productive matmul
    stream.
  - W (4.46MB fp16) is streamed over BOTH hardware DGE queues in stream order:
    sync carries mt[0:4] then the 8 re-halves (rows 0:8 of each out-chunk),
    scalar carries mt[4:8] then the 8 im+mask halves (rows 8:17). Each half
    chunk is one 128-descriptor DMA; a chunk lands just ahead of the matmuls
    that consume it, so the ~16us PE stream and the ~14us (2-queue) DMA fully
    overlap.
  - Main stream per out-chunk j: 8 re matmuls -> ps_r, 8 im matmuls -> ps_i
    (each [128,256], fp32 PSUM). ACT squares ps_r -> fp16, DVE squares ps_i
    and adds, giving one fused prob plane p[j] = re^2+im^2 [128, 256] fp16.
    One Z-mask matmul per j (lhsT [128,32] zero-padded, full rate) accumulates
    all chunks into a single PSUM tile zp [32, 256]; rows 0:10 are the answer.
  - ~18 PE warm-up matmuls on a memset dummy ramp the tensor clock during the
    initial DMA wait.

Sharding: pure data parallel - batch 2048 split as 256 rows per each of the
8 cores; W (fp16, ~4.46MB) replicated per core. Host concatenates per-core
(10, 256) outputs.
"""

import numpy as np

import concourse.bass as bass
import concourse.bacc as bacc
import concourse.mybir as mybir
from concourse.bass_utils import run_bass_kernel_spmd
from concourse.tile import TileContext

NQ = 10
DIM = 1 << NQ          # 1024
BATCH = 2048
NCORES = 8
BPC = BATCH // NCORES  # 256 rows per core
P = 128                # partitions
KC = DIM // P          # in-amp chunks = 8
JC = DIM // P          # out-amp chunks = 8

F32 = mybir.dt.float32
F16 = mybir.dt.float16
MUL = mybir.AluOpType.mult
ADD = mybir.AluOpType.add

LAST_RESULT = None  # BassKernelResults of the most recent run (for test harness)


# ----------------------------------------------------------------------------
# Host-side preprocessing: circuit unitary from weights (numpy, ~2s)
# ----------------------------------------------------------------------------

def _build_circuit_matrix(weights: np.ndarray, dtype=np.complex128) -> np.ndarray:
    """M = U^T: the reference circuit (post-embedding) applied to identity rows."""
    w = weights.astype(np.float64)
    state = np.eye(DIM, dtype=dtype)

    def apply_1q(state, g, q):
        s = state.reshape(DIM, 1 << q, 2, -1)
        s0 = s[:, :, 0, :].copy()
        s1 = s[:, :, 1, :].copy()
        s[:, :, 0, :] = g[0, 0] * s0 + g[0, 1] * s1
        s[:, :, 1, :] = g[1, 0] * s0 + g[1, 1] * s1
        return state

    def apply_2q(state, g4, q1, q2):
        g = g4.reshape(2, 2, 2, 2)
        if q1 > q2:
            g = np.transpose(g, (1, 0, 3, 2))
            q1, q2 = q2, q1
        A = 1 << q1
        M = 1 << (q2 - q1 - 1)
        s = state.reshape(DIM, A, 2, M, 2, -1)
        blocks = [s[:, :, c, :, d, :].copy() for c in (0, 1) for d in (0, 1)]
        for a in (0, 1):
            for b in (0, 1):
                acc = None
                for c in (0, 1):
                    for d in (0, 1):
                        coef = g[a, b, c, d]
                        if coef == 0:
                            continue
                        term = coef * blocks[2 * c + d]
                        acc = term if acc is None else acc + term
                s[:, :, a, :, b, :] = 0 if acc is None else acc
        return state

    def rot_matrix(phi, theta, omega):
        ct, st = np.cos(theta / 2), np.sin(theta / 2)
        return np.array(
            [[np.exp(-0.5j * (phi + omega)) * ct, -np.exp(0.5j * (phi - omega)) * st],
             [np.exp(-0.5j * (phi - omega)) * st, np.exp(0.5j * (phi + omega)) * ct]]
        )

    CNOT = np.array([[1, 0, 0, 0], [0, 1, 0, 0], [0, 0, 0, 1], [0, 0, 1, 0]], dtype)
    I4 = np.eye(4, dtype=dtype)
    XX = np.array([[0, 0, 0, 1], [0, 0, 1, 0], [0, 1, 0, 0], [1, 0, 0, 0]], dtype)
    YY = np.array([[0, 0, 0, -1], [0, 0, 1, 0], [0, 1, 0, 0], [-1, 0, 0, 0]], dtype)

    n_layers = w.shape[0]
    for l in range(n_layers):
        wl = w[l]
        for q in range(NQ):
            state = apply_1q(state, rot_matrix(*wl[q]), q)
        for q in range(NQ):
            state = apply_2q(state, CNOT, q, (q + 1) % NQ)
        c, s_ = np.cos(wl[0, 0] / 2), np.sin(wl[0, 0] / 2)
        state = apply_2q(state, c * I4 + (-1j * s_) * XX, 0, 1)
        c, s_ = np.cos(wl[0, 1] / 2), np.sin(wl[0, 1] / 2)
        state = apply_2q(state, c * I4 + (-1j * s_) * YY, 1, 2)
        e, ec = np.exp(-0.5j * wl[0, 2]), np.exp(0.5j * wl[0, 2])
        state = apply_2q(state, np.diag(np.array([e, ec, ec, e])), 2, 3)
    return state


def _host_prepare(x: np.ndarray, weights: np.ndarray):
    M = _build_circuit_matrix(weights)
    pc = np.array([bin(k).count("1") for k in range(DIM)])
    W = ((-1j) ** pc)[:, None] * M
    wr = W.real.astype(np.float16)   # (1024, 1024) [k, n]
    wi = W.imag.astype(np.float16)

    # wt[j, p, s, c]: j = out-amp chunk, p = in-amp within chunk,
    # s in 0..7 -> (in-chunk ko=s, real), 8..15 -> (ko=s-8, imag),
    # s = 16 -> Z-mask rows: wt[j, p, 16, q] = 1 - 2*bit_q(j*128 + p)
    wr4 = wr.reshape(KC, P, JC, P).transpose(2, 1, 0, 3)  # [j, p, ko, c]
    wi4 = wi.reshape(KC, P, JC, P).transpose(2, 1, 0, 3)
    wt = np.zeros((JC, P, 17, P), dtype=np.float16)
    wt[:, :, 0:8, :] = wr4
    wt[:, :, 8:16, :] = wi4
    n = np.arange(DIM)
    zm = (1 - 2 * ((n[:, None] >> (NQ - 1 - np.arange(NQ))[None, :]) & 1)).astype(
        np.float16
    )  # (1024, 10)
    wt[:, :, 16, :NQ] = zm.reshape(JC, P, NQ)
    wt = np.ascontiguousarray(wt)

    # full embedded state, transposed per core: mt[p, k, b] = m[b, k*128+p]
    xd = x.astype(np.float64)
    c = np.cos(xd / 2)
    s = np.sin(xd / 2)
    B = x.shape[0]
    m = np.ones((B, 1))
    for q in range(NQ):
        f = np.stack([c[:, q], s[:, q]], axis=1)  # (B, 2)
        m = (m[:, :, None] * f[:, None, :]).reshape(B, -1)
    m = m.astype(np.float16)  # (B, 1024), amp bit order: qubit 0 = MSB
    mts = []
    for i in range(NCORES):
        blk = m[i * BPC:(i + 1) * BPC]               # (256, 1024)
        mt = blk.T.reshape(KC, P, BPC).transpose(1, 0, 2)  # [p, k, b]
        mts.append(np.ascontiguousarray(mt))
    return mts, wt


# ----------------------------------------------------------------------------
# Bass kernel (per-core program; SPMD across 8 cores)
# ----------------------------------------------------------------------------

def _build_bass() -> bass.Bass:
    nc = bacc.Bacc(trn_type="TRN2")

    mt_d = nc.dram_tensor("mt", (P, KC, BPC), F16, kind="ExternalInput")
    wt_d = nc.dram_tensor("wt", (JC, P, 17, P), F16, kind="ExternalInput")
    out_d = nc.dram_tensor("out", (NQ, BPC), F32, kind="ExternalOutput")

    with TileContext(nc) as tc:
        with (
            tc.tile_pool(name="wpool", bufs=1) as w_pool,
            tc.tile_pool(name="work", bufs=1) as work_pool,
            tc.tile_pool(name="sq", bufs=3) as sq_pool,
            tc.tile_pool(name="mpsum", bufs=4, space="PSUM") as mpsum,
            tc.tile_pool(name="zpsum", bufs=1, space="PSUM") as zpsum,
            tc.tile_pool(name="wpsum", bufs=1, space="PSUM") as wpsum,
        ):
            # ---- DMA plan: both HWDGE queues, issued in stream order.
            # sync   : mt[0:4] | re-half of each chunk j (rows 0:8)
            # scalar : mt[4:8] | im+mask half of each chunk j (rows 8:17)
            mt_sb = work_pool.tile([P, KC, BPC], F16, name="mt")
            w_sb = [w_pool.tile([P, 17, P], F16, name=f"w_{j}") for j in range(JC)]

            nc.sync.dma_start(mt_sb[:, 0:4, :], mt_d[:, 0:4, :])
            nc.scalar.dma_start(mt_sb[:, 4:8, :], mt_d[:, 4:8, :])
            for j in range(JC):
                nc.sync.dma_start(w_sb[j][:, 0:8, :], wt_d[j, :, 0:8, :])
                nc.scalar.dma_start(w_sb[j][:, 8:17, :], wt_d[j, :, 8:17, :])

            # ---- PE warm-up on a memset dummy: ramps the tensor-engine clock
            # during the initial DMA wait.
            dummy = work_pool.tile([P, BPC], F16, name="dummy")
            nc.vector.memset(dummy, 0.0)
            ps_w = wpsum.tile([P, BPC], F32, name="psw")
            for _ in range(18):
                nc.tensor.matmul(ps_w, lhsT=dummy[:, 0:P], rhs=dummy,
                                 start=True, stop=True)

            # ---- main stream: per out-chunk j, re + im accumulations,
            # fused |psi|^2, and one Z-mask matmul into a single accumulator.
            zp = zpsum.tile([32, BPC], F32, name="zp")
            zout = work_pool.tile([NQ, BPC], F32, name="zout")

            for j in range(JC):
                ps_r = mpsum.tile([P, BPC], F32, name="psr", tag="mmps")
                for k in range(KC):
                    nc.tensor.matmul(
                        ps_r, lhsT=w_sb[j][:, k, :], rhs=mt_sb[:, k, :],
                        start=(k == 0), stop=(k == KC - 1),
                    )
                ps_i = mpsum.tile([P, BPC], F32, name="psi", tag="mmps")
                for k in range(KC):
                    nc.tensor.matmul(
                        ps_i, lhsT=w_sb[j][:, 8 + k, :], rhs=mt_sb[:, k, :],
                        start=(k == 0), stop=(k == KC - 1),
                    )
                sq_r = sq_pool.tile([P, BPC], F16, name="sqr")
                ci = sq_pool.tile([P, BPC], F32, name="ci")
                p_j = sq_pool.tile([P, BPC], F16, name="p")
                nc.scalar.square(sq_r, ps_r)
                nc.vector.tensor_copy(ci, ps_i)
                nc.vector.tensor_tensor(p_j, ci, ci, MUL)
                nc.vector.tensor_tensor(p_j, p_j, sq_r, ADD)
                nc.tensor.matmul(
                    zp, lhsT=w_sb[j][:, 16, 0:32], rhs=p_j,
                    start=(j == 0), stop=(j == JC - 1),
                    skip_group_check=True,
                )

            nc.vector.tensor_copy(zout[:], zp[0:NQ, :])
            nc.sync.dma_start(out_d[:], zout[:])

    nc.finalize()
    return nc


# ----------------------------------------------------------------------------
# Entry point
# ----------------------------------------------------------------------------

def kernel(x: np.ndarray, weights: np.ndarray, _trace: bool = False) -> np.ndarray:
    global LAST_RESULT
    x = np.asarray(x, dtype=np.float32)
    weights = np.asarray(weights, dtype=np.float32)

    mts, wt = _host_prepare(x, weights)

    nc = _build_bass()
    in_maps = [{"mt": mts[i], "wt": wt} for i in range(NCORES)]
    res = run_bass_kernel_spmd(
        nc, in_maps, core_ids=list(range(NCORES)), trace=_trace
    )
    LAST_RESULT = res
    out = np.concatenate(
        [np.asarray(r["out"]).T for r in res.results], axis=0)
    return np.ascontiguousarray(out).astype(np.float32)
